# revision 12
# baseline (speedup 1.0000x reference)
"""Single-head causal attention kernel for Trainium2, 8-core data-parallel.

Problem: x[8, 2048, 1024], w_q/w_k/w_v[64, 1024] (torch Linear convention)
  q = x @ w_q.T; k = x @ w_k.T; v = x @ w_v.T          [B, S, H]
  out = softmax(mask(q @ k.T / sqrt(H))) @ v           [B, S, H]

Sharding: data-parallel over batch, one batch element per NeuronCore.
The host-side shard step also re-lays-out the tensors (pure permutation,
no arithmetic) so the device kernel needs no transposes at all:
  xT_host[p, e, t, s] = x[b][t*128+s, e*128+p]      -> [128, 16384] fp32
  wqk_host[p, e, m]   = concat(w_q, w_k)[m, e*128+p] -> [128, 1024] fp32
  wv_host[p, e, m]    = w_v[m, e*128+p]              -> [128, 512]  fp32

Per-core plan (S=2048, E=1024, H=64):
  - xT loaded fp32->bf16 (SWDGE cast) in a few chunked DMAs, already in
    the [p, e, t, s] transposed layout the matmuls need.
  - pass1 -> [qT; kT] packed (rows 0-63 = qT, 64-127 = kT) [128, 2048];
    kT duplicated onto partitions 0-63 (klow) for the score lhsT.
  - v computed in NATURAL layout [j, 64] via lhsT = xT blocks (M=128,
    N=64) straight into v_all[:, t, 1:65]; col 0 holds ones.
  - scoresT[j, i] = kT_t.T @ qT (K=64, N=512) -> fp32 PSUM; full j-tile
    pairs share a 2-bank [128, 1024] PSUM tile and get a single wide exp
    on ACT (1/8 softmax scale folded in); diagonal tiles get narrowed
    matmuls/exps plus gpsimd affine_select causal masking (fill 0).
  - AV in natural layout: o[i, {den,h}] = sum_t attnT_quarter.T @ v_aug
    (lhsT = attnT [j, 128-i-quarter], rhs = v_all[:, t, :] = [1 | v],
    M=128, N=65, fp32 PSUM accumulate). Column 0 gives the softmax
    denominator; normalize with reciprocal + tensor_scalar_mul on DVE.
  - A few zero matmuls at t=0 warm the PE p-state ramp before real work.
"""

import ml_dtypes
import numpy as np

import concourse.bass as bass
import concourse.bacc as bacc_mod
import concourse.tile as tile
from concourse import mybir
from concourse.bass import ts
from concourse.bass_utils import run_bass_kernel_spmd

B, S, E, H = 8, 2048, 1024, 64
P = 128
NB = S // 512          # 4 column blocks of 512
NT = S // P            # 16 row tiles of 128
ET = E // P            # 8 contraction tiles of 128
FP32 = mybir.dt.float32
BF16 = mybir.dt.bfloat16

N_CORES = 8

# --- schedule knobs -------------------------------------------------------
# HWDGE xT-load chunking: (first_tile, num_tiles)
X_CHUNKS = ((0, 2), (2, 2), (4, 2), (6, 2), (8, 2), (10, 2), (12, 2), (14, 2))
N_WARMUP = 4  # zero matmuls to pin the PE p-state ramp start early


def _emit(nc, tc, ctx, xt_d, wqk_d, wv_d, out_d):
    consts = ctx.enter_context(tc.tile_pool(name="consts", bufs=1))
    sb = ctx.enter_context(tc.tile_pool(name="sb", bufs=1))
    atp = ctx.enter_context(tc.tile_pool(name="atp", bufs=1))
    fin = ctx.enter_context(tc.tile_pool(name="fin", bufs=4))
    psum = ctx.enter_context(tc.tile_pool(name="psum", bufs=2, space="PSUM"))

    # --- PE warmup: zero matmuls while DMAs are in flight ----------------
    wu = consts.tile([P, 512], BF16, tag="warm")
    nc.vector.memset(wu, 0.0)
    for i in range(N_WARMUP):
        ps = psum.tile([P, 512], FP32, tag="proj", name=f"wu{i}")
        nc.tensor.matmul(ps, wu[:, 0:P], wu, start=True, stop=True)

    # --- loads (HWDGE, bf16 pre-cast on host, layout pre-transposed) -----
    # weights first: pass1(0) needs them before anything else
    wqk_all3 = sb.tile([P, ET, P], BF16, tag="wqk")
    nc.sync.dma_start(
        out=wqk_all3, in_=wqk_d.rearrange("p (e m) -> p e m", e=ET)
    )
    wv_all3 = sb.tile([P, ET, H], BF16, tag="wv")
    nc.sync.dma_start(
        out=wv_all3, in_=wv_d.rearrange("p (e m) -> p e m", e=ET)
    )

    xT_all = sb.tile([P, ET * S], BF16, tag="xT_all")
    xT_view = xT_all.rearrange("p (e t s) -> p e t s", e=ET, t=NT)
    xt_dv = xt_d.rearrange("p (e t s) -> p e t s", e=ET, t=NT)
    for t0, k in X_CHUNKS:
        nc.sync.dma_start(
            out=xT_view[:, :, t0:t0 + k, :], in_=xt_dv[:, :, t0:t0 + k, :]
        )

    # --- persistent SBUF tensors -----------------------------------------
    qk1 = sb.tile([P, S], BF16, tag="qk1")     # rows 0-63 qT, 64-127 kT
    klow = sb.tile([H, S], BF16, tag="klow")   # kT on partitions 0-63
    v_all = sb.tile([P, NT, H + 1], BF16, tag="v_all")  # [1 | v] per j-tile
    nc.vector.memset(v_all[:, :, 0:1], 1.0)

    at_pair = {}   # (b, pt) -> [128, 1024] tile: j-tiles 2pt | 2pt+1
    at_diag = {}   # (b, dj) -> [128, 512] tile

    def emit_pass1(nb):
        ps = psum.tile([P, 512], FP32, tag="proj", name=f"p1_{nb}")
        for et in range(ET):
            nc.tensor.matmul(
                ps, wqk_all3[:, et, :],
                xT_view[:, et, 4 * nb:4 * nb + 4, :],
                start=(et == 0), stop=(et == ET - 1),
            )
        nc.vector.tensor_copy(qk1[:, ts(nb, 512)], ps)
        nc.vector.tensor_copy(klow[:, ts(nb, 512)], qk1[H:P, ts(nb, 512)])

    def emit_v(nb):
        for t in range(4 * nb, 4 * nb + 4):
            ps = psum.tile([P, 512], FP32, tag="proj", name=f"v_{t}")
            for et in range(ET):
                nc.tensor.matmul(
                    ps[:, 0:H], xT_view[:, et, t, :], wv_all3[:, et, :],
                    start=(et == 0), stop=(et == ET - 1),
                )
            nc.vector.tensor_copy(v_all[:, t, 1:], ps[:, 0:H])

    def emit_score_pairs(b):
        # full j-tile pairs: two K=64 matmuls into a 2-bank fp32 PSUM tile,
        # evicted by a single wide exp
        for pt in range(2 * b):
            ps = psum.tile([P, 1024], FP32, tag="sc", bufs=3, name=f"s{b}_{pt}")
            for h in range(2):
                t = 2 * pt + h
                nc.tensor.matmul(
                    ps[:, h * 512:(h + 1) * 512],
                    klow[:, ts(t, P)],
                    qk1[0:H, ts(b, 512)],
                    start=True, stop=True,
                )
            at = atp.tile([P, 1024], BF16, tag="pair", bufs=12,
                          name=f"a{b}_{pt}")
            nc.scalar.activation(
                at, ps, mybir.ActivationFunctionType.Exp, scale=0.125,
            )
            at_pair[(b, pt)] = at

    def emit_score_diags(b):
        # diagonal tiles: two per PSUM tile, narrowed width, own exp + mask
        for dh in range(2):
            ps = psum.tile([P, 1024], FP32, tag="sc", bufs=3, name=f"sd{b}_{dh}")
            for h in range(2):
                dj = 2 * dh + h
                t = 4 * b + dj
                c0 = P * dj
                nc.tensor.matmul(
                    ps[:, h * 512 + c0:h * 512 + 512], klow[:, ts(t, P)],
                    qk1[0:H, 512 * b + c0:512 * (b + 1)],
                    start=True, stop=True,
                )
            for h in range(2):
                dj = 2 * dh + h
                c0 = P * dj
                at = atp.tile(
                    [P, 512], BF16, tag="diag", bufs=8, name=f"ad{b}_{dj}"
                )
                nc.scalar.activation(
                    at[:, c0:512], ps[:, h * 512 + c0:h * 512 + 512],
                    mybir.ActivationFunctionType.Exp, scale=0.125,
                )
                nc.gpsimd.affine_select(
                    out=at[:, c0:c0 + P],
                    in_=at[:, c0:c0 + P],
                    compare_op=mybir.AluOpType.is_ge,
                    fill=0.0,
                    base=0,
                    pattern=[[1, P]],
                    channel_multiplier=-1,
                )
                at_diag[(b, dj)] = at

    def _at_slice(b, t, q):
        """attnT[j-tile t, i-quarter q of block b] as a [128, 128] lhsT."""
        if t < 4 * b:
            tile_ = at_pair[(b, t // 2)]
            return tile_[:, (t % 2) * 512 + q * P:(t % 2) * 512 + (q + 1) * P]
        return at_diag[(b, t - 4 * b)][:, q * P:(q + 1) * P]

    def emit_av(b):
        out_sb = fin.tile([P, 4, H], FP32, tag="osb", name=f"osb{b}")
        for q in range(4):
            n_t = 4 * b + q + 1  # causal: j-tiles 0 .. 4b+q
            ps = psum.tile([P, 512], FP32, tag="proj", name=f"av{b}_{q}")
            for t in range(n_t):
                nc.tensor.matmul(
                    ps[:, 0:H + 1], _at_slice(b, t, q), v_all[:, t, :],
                    start=(t == 0), stop=(t == n_t - 1),
                )
            r = fin.tile([P, 1], FP32, tag="recip", name=f"r{b}_{q}")
            nc.vector.reciprocal(r, ps[:, 0:1])
            nc.vector.tensor_scalar_mul(out_sb[:, q, :], ps[:, 1:H + 1], r)
        nc.sync.dma_start(
            out=out_d[ts(b, 512), :].rearrange("(q p) h -> p q h", p=P),
            in_=out_sb,
        )

    # --- main software-pipelined loop -------------------------------------
    for nb in range(NB):
        emit_pass1(nb)
        emit_score_pairs(nb)
        emit_v(nb)
        if nb >= 1:
            emit_av(nb - 1)
        emit_score_diags(nb)
    emit_av(NB - 1)


_NC_CACHE = {}


def _build_nc():
    if "nc" not in _NC_CACHE:
        from contextlib import ExitStack

        nc = bacc_mod.Bacc("TRN2")
        xt_d = nc.dram_tensor("xt", [P, ET * S], BF16, kind="ExternalInput")
        wqk_d = nc.dram_tensor("wqk", [P, ET * P], BF16, kind="ExternalInput")
        wv_d = nc.dram_tensor("wv", [P, ET * H], BF16, kind="ExternalInput")
        out_d = nc.dram_tensor("out", [S, H], FP32, kind="ExternalOutput")
        with tile.TileContext(nc) as tc:
            with ExitStack() as ctx:
                _emit(nc, tc, ctx, xt_d[:, :], wqk_d[:, :], wv_d[:, :],
                      out_d[:, :])
        nc.compile()
        _NC_CACHE["nc"] = nc
    return _NC_CACHE["nc"]


def _relayout_e_major(w):
    """[M, E] fp32 -> bf16 [128, ET*M]: out[p, e*M + m] = w[m, e*128 + p]."""
    m = w.shape[0]
    return np.ascontiguousarray(
        w.T.reshape(ET, P, m).transpose(1, 0, 2).reshape(P, ET * m)
    ).astype(ml_dtypes.bfloat16)


def kernel(x, w_q, w_k, w_v, _trace=False, _trace_kwargs=None):
    nc = _build_nc()
    x = np.ascontiguousarray(x, dtype=np.float32)
    # host-side layout permutations (no arithmetic): see module docstring
    wqk_host = _relayout_e_major(
        np.concatenate(
            [np.asarray(w_q, np.float32), np.asarray(w_k, np.float32)], axis=0
        )
    )
    wv_host = _relayout_e_major(np.asarray(w_v, np.float32))
    in_maps = []
    for b in range(N_CORES):
        # xt[p, e, t, s] = x[b][t*128+s, e*128+p], pre-cast to bf16
        xt = np.ascontiguousarray(
            x[b].reshape(NT, P, ET, P).transpose(3, 2, 0, 1).reshape(P, ET * S)
        ).astype(ml_dtypes.bfloat16)
        in_maps.append({"xt": xt, "wqk": wqk_host, "wv": wv_host})
    res = run_bass_kernel_spmd(
        nc, in_maps, list(range(N_CORES)), trace=_trace,
        **(_trace_kwargs or {}),
    )
    out = np.stack([res.results[b]["out"] for b in range(N_CORES)])
    if _trace:
        return out.astype(np.float32), res
    return out.astype(np.float32)


# revision 13
# speedup vs baseline: 1.1358x; 1.1358x over previous
"""Single-head causal attention kernel for Trainium2, 8-core data-parallel.

Problem: x[8, 2048, 1024], w_q/w_k/w_v[64, 1024] (torch Linear convention)
  q = x @ w_q.T; k = x @ w_k.T; v = x @ w_v.T          [B, S, H]
  out = softmax(mask(q @ k.T / sqrt(H))) @ v           [B, S, H]

Sharding: data-parallel over batch, one batch element per NeuronCore.
The host-side shard step also re-lays-out the tensors (pure permutation,
no arithmetic) so the device kernel needs no transposes at all:
  xT_host[p, e, t, s] = x[b][t*128+s, e*128+p]      -> [128, 16384] fp32
  wqk_host[p, e, m]   = concat(w_q, w_k)[m, e*128+p] -> [128, 1024] fp32
  wv_host[p, e, m]    = w_v[m, e*128+p]              -> [128, 512]  fp32

Per-core plan (S=2048, E=1024, H=64):
  - xT loaded fp32->bf16 (SWDGE cast) in a few chunked DMAs, already in
    the [p, e, t, s] transposed layout the matmuls need.
  - pass1 -> [qT; kT] packed (rows 0-63 = qT, 64-127 = kT) [128, 2048];
    kT duplicated onto partitions 0-63 (klow) for the score lhsT.
  - v computed in NATURAL layout [j, 64] via lhsT = xT blocks (M=128,
    N=64) straight into v_all[:, t, 1:65]; col 0 holds ones.
  - scoresT[j, i] = kT_t.T @ qT (K=64, N=512) -> fp32 PSUM; full j-tile
    pairs share a 2-bank [128, 1024] PSUM tile and get a single wide exp
    on ACT (1/8 softmax scale folded in); diagonal tiles get narrowed
    matmuls/exps plus gpsimd affine_select causal masking (fill 0).
  - AV in natural layout: o[i, {den,h}] = sum_t attnT_quarter.T @ v_aug
    (lhsT = attnT [j, 128-i-quarter], rhs = v_all[:, t, :] = [1 | v],
    M=128, N=65, fp32 PSUM accumulate). Column 0 gives the softmax
    denominator; normalize with reciprocal + tensor_scalar_mul on DVE.
  - A few zero matmuls at t=0 warm the PE p-state ramp before real work.
"""

import ml_dtypes
import numpy as np

import concourse.bass as bass
import concourse.bacc as bacc_mod
import concourse.tile as tile
from concourse import mybir
from concourse.bass import ts
from concourse.bass_utils import run_bass_kernel_spmd

B, S, E, H = 8, 2048, 1024, 64
P = 128
NB = S // 512          # 4 column blocks of 512
NT = S // P            # 16 row tiles of 128
ET = E // P            # 8 contraction tiles of 128
FP32 = mybir.dt.float32
BF16 = mybir.dt.bfloat16

N_CORES = 8

# --- schedule knobs -------------------------------------------------------
# HWDGE xT-load chunking: (first_tile, num_tiles)
X_CHUNKS = ((0, 2), (2, 2), (4, 2), (6, 2), (8, 2), (10, 2), (12, 2), (14, 2))
N_WARMUP = 10  # zero matmuls to bridge the PE p-state ramp until data


def _emit(nc, tc, ctx, xt_d, wqk_d, wv_d, out_d):
    consts = ctx.enter_context(tc.tile_pool(name="consts", bufs=1))
    sb = ctx.enter_context(tc.tile_pool(name="sb", bufs=1))
    atp = ctx.enter_context(tc.tile_pool(name="atp", bufs=1))
    fin = ctx.enter_context(tc.tile_pool(name="fin", bufs=4))
    psum = ctx.enter_context(tc.tile_pool(name="psum", bufs=2, space="PSUM"))

    # --- PE warmup: zero matmuls while DMAs are in flight ----------------
    wu = consts.tile([P, 512], BF16, tag="warm")
    nc.vector.memset(wu, 0.0)
    for i in range(N_WARMUP):
        ps = psum.tile([P, 512], FP32, tag="proj", name=f"wu{i}")
        nc.tensor.matmul(ps, wu[:, 0:P], wu, start=True, stop=True)

    # --- loads (HWDGE, bf16 pre-cast on host, layout pre-transposed) -----
    # order: first two x chunks, weights, remaining x chunks — minimizes
    # the critical chain into pass1(0)
    xT_all = sb.tile([P, ET * S], BF16, tag="xT_all")
    xT_view = xT_all.rearrange("p (e t s) -> p e t s", e=ET, t=NT)
    xt_dv = xt_d.rearrange("p (e t s) -> p e t s", e=ET, t=NT)

    def emit_x_chunk(t0, k):
        nc.sync.dma_start(
            out=xT_view[:, :, t0:t0 + k, :], in_=xt_dv[:, :, t0:t0 + k, :]
        )

    for t0, k in X_CHUNKS[:2]:
        emit_x_chunk(t0, k)
    wqk_all3 = sb.tile([P, ET, P], BF16, tag="wqk")
    nc.sync.dma_start(
        out=wqk_all3, in_=wqk_d.rearrange("p (e m) -> p e m", e=ET)
    )
    wv_all3 = sb.tile([P, ET, H], BF16, tag="wv")
    nc.sync.dma_start(
        out=wv_all3, in_=wv_d.rearrange("p (e m) -> p e m", e=ET)
    )
    for t0, k in X_CHUNKS[2:]:
        emit_x_chunk(t0, k)

    # --- persistent SBUF tensors -----------------------------------------
    qk1 = sb.tile([P, S], BF16, tag="qk1")     # rows 0-63 qT, 64-127 kT
    klow = sb.tile([H, S], BF16, tag="klow")   # kT on partitions 0-63
    v_all = sb.tile([P, NT, H + 1], BF16, tag="v_all")  # [1 | v] per j-tile
    nc.vector.memset(v_all[:, :, 0:1], 1.0)

    at_pair = {}   # (b, pt) -> [128, 1024] tile: j-tiles 2pt | 2pt+1
    at_diag = {}   # (b, dj) -> [128, 512] tile

    def emit_pass1(nb):
        ps = psum.tile([P, 512], FP32, tag="proj", name=f"p1_{nb}")
        for et in range(ET):
            nc.tensor.matmul(
                ps, wqk_all3[:, et, :],
                xT_view[:, et, 4 * nb:4 * nb + 4, :],
                start=(et == 0), stop=(et == ET - 1),
            )
        nc.vector.tensor_copy(qk1[:, ts(nb, 512)], ps)
        nc.vector.tensor_copy(klow[:, ts(nb, 512)], qk1[H:P, ts(nb, 512)])

    def emit_v(nb):
        for t in range(4 * nb, 4 * nb + 4):
            ps = psum.tile([P, 512], FP32, tag="proj", name=f"v_{t}")
            for et in range(ET):
                nc.tensor.matmul(
                    ps[:, 0:H], xT_view[:, et, t, :], wv_all3[:, et, :],
                    start=(et == 0), stop=(et == ET - 1),
                )
            nc.vector.tensor_copy(v_all[:, t, 1:], ps[:, 0:H])

    def emit_score_pairs(b):
        # full j-tile pairs: two K=64 matmuls into a 2-bank fp32 PSUM tile,
        # evicted by a single wide exp
        for pt in range(2 * b):
            ps = psum.tile([P, 1024], FP32, tag="sc", bufs=3, name=f"s{b}_{pt}")
            for h in range(2):
                t = 2 * pt + h
                nc.tensor.matmul(
                    ps[:, h * 512:(h + 1) * 512],
                    klow[:, ts(t, P)],
                    qk1[0:H, ts(b, 512)],
                    start=True, stop=True,
                )
            at = atp.tile([P, 1024], BF16, tag="pair", bufs=12,
                          name=f"a{b}_{pt}")
            nc.scalar.activation(
                at, ps, mybir.ActivationFunctionType.Exp, scale=0.125,
            )
            at_pair[(b, pt)] = at

    def emit_score_diags(b):
        # diagonal tiles: two per PSUM tile, narrowed width, own exp + mask
        for dh in range(2):
            ps = psum.tile([P, 1024], FP32, tag="sc", bufs=3, name=f"sd{b}_{dh}")
            for h in range(2):
                dj = 2 * dh + h
                t = 4 * b + dj
                c0 = P * dj
                nc.tensor.matmul(
                    ps[:, h * 512 + c0:h * 512 + 512], klow[:, ts(t, P)],
                    qk1[0:H, 512 * b + c0:512 * (b + 1)],
                    start=True, stop=True,
                )
            for h in range(2):
                dj = 2 * dh + h
                c0 = P * dj
                at = atp.tile(
                    [P, 512], BF16, tag="diag", bufs=8, name=f"ad{b}_{dj}"
                )
                nc.scalar.activation(
                    at[:, c0:512], ps[:, h * 512 + c0:h * 512 + 512],
                    mybir.ActivationFunctionType.Exp, scale=0.125,
                )
                nc.gpsimd.affine_select(
                    out=at[:, c0:c0 + P],
                    in_=at[:, c0:c0 + P],
                    compare_op=mybir.AluOpType.is_ge,
                    fill=0.0,
                    base=0,
                    pattern=[[1, P]],
                    channel_multiplier=-1,
                )
                at_diag[(b, dj)] = at

    def _at_slice(b, t, q):
        """attnT[j-tile t, i-quarter q of block b] as a [128, 128] lhsT."""
        if t < 4 * b:
            tile_ = at_pair[(b, t // 2)]
            return tile_[:, (t % 2) * 512 + q * P:(t % 2) * 512 + (q + 1) * P]
        return at_diag[(b, t - 4 * b)][:, q * P:(q + 1) * P]

    def emit_av(b):
        out_sb = fin.tile([P, 4, H], FP32, tag="osb", name=f"osb{b}")
        for q in range(4):
            n_t = 4 * b + q + 1  # causal: j-tiles 0 .. 4b+q
            ps = psum.tile([P, 512], FP32, tag="proj", name=f"av{b}_{q}")
            for t in range(n_t):
                nc.tensor.matmul(
                    ps[:, 0:H + 1], _at_slice(b, t, q), v_all[:, t, :],
                    start=(t == 0), stop=(t == n_t - 1),
                )
            r = fin.tile([P, 1], FP32, tag="recip", name=f"r{b}_{q}")
            nc.vector.reciprocal(r, ps[:, 0:1])
            nc.vector.tensor_scalar_mul(out_sb[:, q, :], ps[:, 1:H + 1], r)
        nc.sync.dma_start(
            out=out_d[ts(b, 512), :].rearrange("(q p) h -> p q h", p=P),
            in_=out_sb,
        )

    # --- main software-pipelined loop -------------------------------------
    for nb in range(NB):
        emit_pass1(nb)
        emit_score_pairs(nb)
        emit_v(nb)
        if nb >= 1:
            emit_av(nb - 1)
        emit_score_diags(nb)
    emit_av(NB - 1)


_NC_CACHE = {}


def _build_nc():
    if "nc" not in _NC_CACHE:
        from contextlib import ExitStack

        nc = bacc_mod.Bacc("TRN2")
        xt_d = nc.dram_tensor("xt", [P, ET * S], BF16, kind="ExternalInput")
        wqk_d = nc.dram_tensor("wqk", [P, ET * P], BF16, kind="ExternalInput")
        wv_d = nc.dram_tensor("wv", [P, ET * H], BF16, kind="ExternalInput")
        out_d = nc.dram_tensor("out", [S, H], FP32, kind="ExternalOutput")
        with tile.TileContext(nc) as tc:
            with ExitStack() as ctx:
                _emit(nc, tc, ctx, xt_d[:, :], wqk_d[:, :], wv_d[:, :],
                      out_d[:, :])
        nc.compile()
        _NC_CACHE["nc"] = nc
    return _NC_CACHE["nc"]


def _relayout_e_major(w):
    """[M, E] fp32 -> bf16 [128, ET*M]: out[p, e*M + m] = w[m, e*128 + p]."""
    m = w.shape[0]
    return np.ascontiguousarray(
        w.T.reshape(ET, P, m).transpose(1, 0, 2).reshape(P, ET * m)
    ).astype(ml_dtypes.bfloat16)


def kernel(x, w_q, w_k, w_v, _trace=False, _trace_kwargs=None):
    nc = _build_nc()
    x = np.ascontiguousarray(x, dtype=np.float32)
    # host-side layout permutations (no arithmetic): see module docstring
    wqk_host = _relayout_e_major(
        np.concatenate(
            [np.asarray(w_q, np.float32), np.asarray(w_k, np.float32)], axis=0
        )
    )
    wv_host = _relayout_e_major(np.asarray(w_v, np.float32))
    in_maps = []
    for b in range(N_CORES):
        # xt[p, e, t, s] = x[b][t*128+s, e*128+p], pre-cast to bf16
        xt = np.ascontiguousarray(
            x[b].reshape(NT, P, ET, P).transpose(3, 2, 0, 1).reshape(P, ET * S)
        ).astype(ml_dtypes.bfloat16)
        in_maps.append({"xt": xt, "wqk": wqk_host, "wv": wv_host})
    res = run_bass_kernel_spmd(
        nc, in_maps, list(range(N_CORES)), trace=_trace,
        **(_trace_kwargs or {}),
    )
    out = np.stack([res.results[b]["out"] for b in range(N_CORES)])
    if _trace:
        return out.astype(np.float32), res
    return out.astype(np.float32)


# revision 14
# speedup vs baseline: 1.1827x; 1.0412x over previous
"""Single-head causal attention kernel for Trainium2, 8-core data-parallel.

Problem: x[8, 2048, 1024], w_q/w_k/w_v[64, 1024] (torch Linear convention)
  q = x @ w_q.T; k = x @ w_k.T; v = x @ w_v.T          [B, S, H]
  out = softmax(mask(q @ k.T / sqrt(H))) @ v           [B, S, H]

Sharding: data-parallel over batch, one batch element per NeuronCore.
The host-side shard step also re-lays-out the tensors (pure permutation,
no arithmetic) so the device kernel needs no transposes at all:
  xT_host[p, e, t, s] = x[b][t*128+s, e*128+p]      -> [128, 16384] fp32
  wqk_host[p, e, m]   = concat(w_q, w_k)[m, e*128+p] -> [128, 1024] fp32
  wv_host[p, e, m]    = w_v[m, e*128+p]              -> [128, 512]  fp32

Per-core plan (S=2048, E=1024, H=64):
  - xT loaded fp32->bf16 (SWDGE cast) in a few chunked DMAs, already in
    the [p, e, t, s] transposed layout the matmuls need.
  - pass1 -> [qT; kT] packed (rows 0-63 = qT, 64-127 = kT) [128, 2048];
    kT duplicated onto partitions 0-63 (klow) for the score lhsT.
  - v computed in NATURAL layout [j, 64] via lhsT = xT blocks (M=128,
    N=64) straight into v_all[:, t, 1:65]; col 0 holds ones.
  - scoresT[j, i] = kT_t.T @ qT (K=64, N=512) -> fp32 PSUM; full j-tile
    pairs share a 2-bank [128, 1024] PSUM tile and get a single wide exp
    on ACT (1/8 softmax scale folded in); diagonal tiles get narrowed
    matmuls/exps plus gpsimd affine_select causal masking (fill 0).
  - AV in natural layout: o[i, {den,h}] = sum_t attnT_quarter.T @ v_aug
    (lhsT = attnT [j, 128-i-quarter], rhs = v_all[:, t, :] = [1 | v],
    M=128, N=65, fp32 PSUM accumulate). Column 0 gives the softmax
    denominator; normalize with reciprocal + tensor_scalar_mul on DVE.
  - A few zero matmuls at t=0 warm the PE p-state ramp before real work.
"""

import ml_dtypes
import numpy as np

import concourse.bass as bass
import concourse.bacc as bacc_mod
import concourse.tile as tile
from concourse import mybir
from concourse.bass import ts
from concourse.bass_utils import run_bass_kernel_spmd

B, S, E, H = 8, 2048, 1024, 64
P = 128
NB = S // 512          # 4 column blocks of 512
NT = S // P            # 16 row tiles of 128
ET = E // P            # 8 contraction tiles of 128
FP32 = mybir.dt.float32
BF16 = mybir.dt.bfloat16

N_CORES = 8

# --- schedule knobs -------------------------------------------------------
# HWDGE xT-load chunking: (first_tile, num_tiles)
X_CHUNKS = ((0, 2), (2, 2), (4, 2), (6, 2), (8, 2), (10, 2), (12, 2), (14, 2))
N_WARMUP = 10  # zero matmuls to bridge the PE p-state ramp until data


def _emit(nc, tc, ctx, xt_d, wqk_d, wv_d, out_d):
    consts = ctx.enter_context(tc.tile_pool(name="consts", bufs=1))
    sb = ctx.enter_context(tc.tile_pool(name="sb", bufs=1))
    atp = ctx.enter_context(tc.tile_pool(name="atp", bufs=1))
    fin = ctx.enter_context(tc.tile_pool(name="fin", bufs=4))
    psum = ctx.enter_context(tc.tile_pool(name="psum", bufs=2, space="PSUM"))

    # --- PE warmup: zero matmuls while DMAs are in flight ----------------
    wu = consts.tile([P, 512], BF16, tag="warm")
    nc.vector.memset(wu, 0.0)
    for i in range(N_WARMUP):
        ps = psum.tile([P, 512], FP32, tag="proj", name=f"wu{i}")
        nc.tensor.matmul(ps, wu[:, 0:P], wu, start=True, stop=True)

    # --- loads (HWDGE, bf16 pre-cast on host, layout pre-transposed) -----
    # order: first two x chunks, weights, remaining x chunks — minimizes
    # the critical chain into pass1(0)
    xT_all = sb.tile([P, ET * S], BF16, tag="xT_all")
    xT_view = xT_all.rearrange("p (e t s) -> p e t s", e=ET, t=NT)
    xt_dv = xt_d.rearrange("p (e t s) -> p e t s", e=ET, t=NT)

    def emit_x_chunk(t0, k):
        nc.sync.dma_start(
            out=xT_view[:, :, t0:t0 + k, :], in_=xt_dv[:, :, t0:t0 + k, :]
        )

    for t0, k in X_CHUNKS[:2]:
        emit_x_chunk(t0, k)
    wqk_all3 = sb.tile([P, ET, P], BF16, tag="wqk")
    nc.sync.dma_start(
        out=wqk_all3, in_=wqk_d.rearrange("p (e m) -> p e m", e=ET)
    )
    wv_all3 = sb.tile([P, ET, H], BF16, tag="wv")
    nc.sync.dma_start(
        out=wv_all3, in_=wv_d.rearrange("p (e m) -> p e m", e=ET)
    )
    for t0, k in X_CHUNKS[2:]:
        emit_x_chunk(t0, k)

    # --- persistent SBUF tensors -----------------------------------------
    qk1 = sb.tile([P, S], BF16, tag="qk1")     # rows 0-63 qT, 64-127 kT
    klow = sb.tile([H, S], BF16, tag="klow")   # kT on partitions 0-63
    v_all = sb.tile([P, NT, H + 1], BF16, tag="v_all")  # [1 | v] per j-tile
    nc.vector.memset(v_all[:, :, 0:1], 1.0)

    at_pair = {}   # (b, pt) -> [128, 1024] tile: j-tiles 2pt | 2pt+1
    at_diag = {}   # (b, dj) -> [128, 512] tile

    def emit_pass1(nb):
        ps = psum.tile([P, 512], FP32, tag="proj", name=f"p1_{nb}")
        for et in range(ET):
            nc.tensor.matmul(
                ps, wqk_all3[:, et, :],
                xT_view[:, et, 4 * nb:4 * nb + 4, :],
                start=(et == 0), stop=(et == ET - 1),
            )
        nc.vector.tensor_copy(qk1[:, ts(nb, 512)], ps)
        nc.vector.tensor_copy(klow[:, ts(nb, 512)], qk1[H:P, ts(nb, 512)])

    def emit_v(t_lo, t_hi):
        for t in range(t_lo, t_hi):
            ps = psum.tile([P, 512], FP32, tag="proj", name=f"v_{t}")
            for et in range(ET):
                nc.tensor.matmul(
                    ps[:, 0:H], xT_view[:, et, t, :], wv_all3[:, et, :],
                    start=(et == 0), stop=(et == ET - 1),
                )
            nc.vector.tensor_copy(v_all[:, t, 1:], ps[:, 0:H])

    def emit_score_pairs(b):
        # full j-tile pairs: two K=64 matmuls into a 2-bank fp32 PSUM tile,
        # evicted by a single wide exp
        for pt in range(2 * b):
            ps = psum.tile([P, 1024], FP32, tag="sc", bufs=3, name=f"s{b}_{pt}")
            for h in range(2):
                t = 2 * pt + h
                nc.tensor.matmul(
                    ps[:, h * 512:(h + 1) * 512],
                    klow[:, ts(t, P)],
                    qk1[0:H, ts(b, 512)],
                    start=True, stop=True,
                )
            at = atp.tile([P, 1024], BF16, tag="pair", bufs=12,
                          name=f"a{b}_{pt}")
            nc.scalar.activation(
                at, ps, mybir.ActivationFunctionType.Exp, scale=0.125,
            )
            at_pair[(b, pt)] = at

    def emit_score_diags(b):
        # diagonal tiles: two per PSUM tile, narrowed width, own exp + mask
        for dh in range(2):
            ps = psum.tile([P, 1024], FP32, tag="sc", bufs=3, name=f"sd{b}_{dh}")
            for h in range(2):
                dj = 2 * dh + h
                t = 4 * b + dj
                c0 = P * dj
                nc.tensor.matmul(
                    ps[:, h * 512 + c0:h * 512 + 512], klow[:, ts(t, P)],
                    qk1[0:H, 512 * b + c0:512 * (b + 1)],
                    start=True, stop=True,
                )
            for h in range(2):
                dj = 2 * dh + h
                c0 = P * dj
                at = atp.tile(
                    [P, 512], BF16, tag="diag", bufs=16, name=f"ad{b}_{dj}"
                )
                nc.scalar.activation(
                    at[:, c0:512], ps[:, h * 512 + c0:h * 512 + 512],
                    mybir.ActivationFunctionType.Exp, scale=0.125,
                )
                nc.gpsimd.affine_select(
                    out=at[:, c0:c0 + P],
                    in_=at[:, c0:c0 + P],
                    compare_op=mybir.AluOpType.is_ge,
                    fill=0.0,
                    base=0,
                    pattern=[[1, P]],
                    channel_multiplier=-1,
                )
                at_diag[(b, dj)] = at

    def _at_slice(b, t, q):
        """attnT[j-tile t, i-quarter q of block b] as a [128, 128] lhsT."""
        if t < 4 * b:
            tile_ = at_pair[(b, t // 2)]
            return tile_[:, (t % 2) * 512 + q * P:(t % 2) * 512 + (q + 1) * P]
        return at_diag[(b, t - 4 * b)][:, q * P:(q + 1) * P]

    def emit_av(b, split_out=False):
        out_sb = fin.tile([P, 4, H], FP32, tag="osb", name=f"osb{b}")
        for q in range(4):
            n_t = 4 * b + q + 1  # causal: j-tiles 0 .. 4b+q
            ps = psum.tile([P, 512], FP32, tag="proj", name=f"av{b}_{q}")
            for t in range(n_t):
                nc.tensor.matmul(
                    ps[:, 0:H + 1], _at_slice(b, t, q), v_all[:, t, :],
                    start=(t == 0), stop=(t == n_t - 1),
                )
            r = fin.tile([P, 1], FP32, tag="recip", name=f"r{b}_{q}")
            nc.vector.reciprocal(r, ps[:, 0:1])
            nc.vector.tensor_scalar_mul(out_sb[:, q, :], ps[:, 1:H + 1], r)
            if split_out and q >= 2:
                nc.sync.dma_start(
                    out=out_d[512 * b + 128 * q:512 * b + 128 * (q + 1), :],
                    in_=out_sb[:, q, :],
                )
        if split_out:
            nc.sync.dma_start(
                out=out_d[512 * b:512 * b + 256, :].rearrange(
                    "(q p) h -> p q h", p=P
                ),
                in_=out_sb[:, 0:2, :],
            )
        else:
            nc.sync.dma_start(
                out=out_d[ts(b, 512), :].rearrange("(q p) h -> p q h", p=P),
                in_=out_sb,
            )

    # --- main software-pipelined schedule ---------------------------------
    # prioritize the pass1 -> scores chain (ACT's exp feed); v/AV fill the
    # PE while ACT drains, diag exps stay last per block so AV can stagger
    emit_pass1(0)
    emit_v(0, 2)
    emit_score_diags(0)
    emit_pass1(1)
    emit_score_pairs(1)
    emit_score_diags(1)
    emit_v(2, 4)
    emit_v(4, 8)
    emit_pass1(2)
    emit_score_pairs(2)
    emit_score_diags(2)
    emit_pass1(3)
    emit_score_pairs(3)
    emit_score_diags(3)
    emit_av(0)
    emit_v(8, 12)
    emit_v(12, 16)
    emit_av(1)
    emit_av(2)
    emit_av(3, split_out=True)


_NC_CACHE = {}


def _build_nc():
    if "nc" not in _NC_CACHE:
        from contextlib import ExitStack

        nc = bacc_mod.Bacc("TRN2")
        xt_d = nc.dram_tensor("xt", [P, ET * S], BF16, kind="ExternalInput")
        wqk_d = nc.dram_tensor("wqk", [P, ET * P], BF16, kind="ExternalInput")
        wv_d = nc.dram_tensor("wv", [P, ET * H], BF16, kind="ExternalInput")
        out_d = nc.dram_tensor("out", [S, H], FP32, kind="ExternalOutput")
        with tile.TileContext(nc) as tc:
            with ExitStack() as ctx:
                _emit(nc, tc, ctx, xt_d[:, :], wqk_d[:, :], wv_d[:, :],
                      out_d[:, :])
        nc.compile()
        _NC_CACHE["nc"] = nc
    return _NC_CACHE["nc"]


def _relayout_e_major(w):
    """[M, E] fp32 -> bf16 [128, ET*M]: out[p, e*M + m] = w[m, e*128 + p]."""
    m = w.shape[0]
    return np.ascontiguousarray(
        w.T.reshape(ET, P, m).transpose(1, 0, 2).reshape(P, ET * m)
    ).astype(ml_dtypes.bfloat16)


def kernel(x, w_q, w_k, w_v, _trace=False, _trace_kwargs=None):
    nc = _build_nc()
    x = np.ascontiguousarray(x, dtype=np.float32)
    # host-side layout permutations (no arithmetic): see module docstring
    wqk_host = _relayout_e_major(
        np.concatenate(
            [np.asarray(w_q, np.float32), np.asarray(w_k, np.float32)], axis=0
        )
    )
    wv_host = _relayout_e_major(np.asarray(w_v, np.float32))
    in_maps = []
    for b in range(N_CORES):
        # xt[p, e, t, s] = x[b][t*128+s, e*128+p], pre-cast to bf16
        xt = np.ascontiguousarray(
            x[b].reshape(NT, P, ET, P).transpose(3, 2, 0, 1).reshape(P, ET * S)
        ).astype(ml_dtypes.bfloat16)
        in_maps.append({"xt": xt, "wqk": wqk_host, "wv": wv_host})
    res = run_bass_kernel_spmd(
        nc, in_maps, list(range(N_CORES)), trace=_trace,
        **(_trace_kwargs or {}),
    )
    out = np.stack([res.results[b]["out"] for b in range(N_CORES)])
    if _trace:
        return out.astype(np.float32), res
    return out.astype(np.float32)


# revision 15
# speedup vs baseline: 1.1895x; 1.0058x over previous
"""Single-head causal attention kernel for Trainium2, 8-core data-parallel.

Problem: x[8, 2048, 1024], w_q/w_k/w_v[64, 1024] (torch Linear convention)
  q = x @ w_q.T; k = x @ w_k.T; v = x @ w_v.T          [B, S, H]
  out = softmax(mask(q @ k.T / sqrt(H))) @ v           [B, S, H]

Sharding: data-parallel over batch, one batch element per NeuronCore.
The host-side shard step also re-lays-out the tensors (pure permutation,
no arithmetic) so the device kernel needs no transposes at all:
  xT_host[p, e, t, s] = x[b][t*128+s, e*128+p]      -> [128, 16384] fp32
  wqk_host[p, e, m]   = concat(w_q, w_k)[m, e*128+p] -> [128, 1024] fp32
  wv_host[p, e, m]    = w_v[m, e*128+p]              -> [128, 512]  fp32

Per-core plan (S=2048, E=1024, H=64):
  - xT loaded fp32->bf16 (SWDGE cast) in a few chunked DMAs, already in
    the [p, e, t, s] transposed layout the matmuls need.
  - pass1 -> [qT; kT] packed (rows 0-63 = qT, 64-127 = kT) [128, 2048];
    kT duplicated onto partitions 0-63 (klow) for the score lhsT.
  - v computed in NATURAL layout [j, 64] via lhsT = xT blocks (M=128,
    N=64) straight into v_all[:, t, 1:65]; col 0 holds ones.
  - scoresT[j, i] = kT_t.T @ qT (K=64, N=512) -> fp32 PSUM; full j-tile
    pairs share a 2-bank [128, 1024] PSUM tile and get a single wide exp
    on ACT (1/8 softmax scale folded in); diagonal tiles get narrowed
    matmuls/exps plus gpsimd affine_select causal masking (fill 0).
  - AV in natural layout: o[i, {den,h}] = sum_t attnT_quarter.T @ v_aug
    (lhsT = attnT [j, 128-i-quarter], rhs = v_all[:, t, :] = [1 | v],
    M=128, N=65, fp32 PSUM accumulate). Column 0 gives the softmax
    denominator; normalize with reciprocal + tensor_scalar_mul on DVE.
  - A few zero matmuls at t=0 warm the PE p-state ramp before real work.
"""

import ml_dtypes
import numpy as np

import concourse.bass as bass
import concourse.bacc as bacc_mod
import concourse.tile as tile
from concourse import mybir
from concourse.bass import ts
from concourse.bass_utils import run_bass_kernel_spmd

B, S, E, H = 8, 2048, 1024, 64
P = 128
NB = S // 512          # 4 column blocks of 512
NT = S // P            # 16 row tiles of 128
ET = E // P            # 8 contraction tiles of 128
FP32 = mybir.dt.float32
BF16 = mybir.dt.bfloat16

N_CORES = 8

# --- schedule knobs -------------------------------------------------------
# HWDGE xT-load chunking: (first_tile, num_tiles)
X_CHUNKS = ((0, 2), (2, 2), (4, 2), (6, 2), (8, 2), (10, 2), (12, 2), (14, 2))
N_WARMUP = 7  # zero matmuls to bridge the PE p-state ramp until data


def _emit(nc, tc, ctx, xt_d, wqk_d, wv_d, out_d):
    consts = ctx.enter_context(tc.tile_pool(name="consts", bufs=1))
    sb = ctx.enter_context(tc.tile_pool(name="sb", bufs=1))
    atp = ctx.enter_context(tc.tile_pool(name="atp", bufs=1))
    fin = ctx.enter_context(tc.tile_pool(name="fin", bufs=4))
    psum = ctx.enter_context(tc.tile_pool(name="psum", bufs=2, space="PSUM"))

    # --- PE warmup: zero matmuls while DMAs are in flight ----------------
    wu = consts.tile([P, 512], BF16, tag="warm")
    nc.vector.memset(wu, 0.0)
    for i in range(N_WARMUP):
        ps = psum.tile([P, 512], FP32, tag="proj", name=f"wu{i}")
        nc.tensor.matmul(ps, wu[:, 0:P], wu, start=True, stop=True)

    # --- loads (HWDGE, bf16 pre-cast on host, layout pre-transposed) -----
    # order: first two x chunks, weights, remaining x chunks — minimizes
    # the critical chain into pass1(0)
    xT_all = sb.tile([P, ET * S], BF16, tag="xT_all")
    xT_view = xT_all.rearrange("p (e t s) -> p e t s", e=ET, t=NT)
    xt_dv = xt_d.rearrange("p (e t s) -> p e t s", e=ET, t=NT)

    def emit_x_chunk(t0, k):
        nc.sync.dma_start(
            out=xT_view[:, :, t0:t0 + k, :], in_=xt_dv[:, :, t0:t0 + k, :]
        )

    for t0, k in X_CHUNKS[:2]:
        emit_x_chunk(t0, k)
    wqk_all3 = sb.tile([P, ET, P], BF16, tag="wqk")
    nc.sync.dma_start(
        out=wqk_all3, in_=wqk_d.rearrange("p (e m) -> p e m", e=ET)
    )
    wv_all3 = sb.tile([P, ET, H], BF16, tag="wv")
    nc.sync.dma_start(
        out=wv_all3, in_=wv_d.rearrange("p (e m) -> p e m", e=ET)
    )
    for t0, k in X_CHUNKS[2:]:
        emit_x_chunk(t0, k)

    # --- persistent SBUF tensors -----------------------------------------
    qk1 = sb.tile([P, S], BF16, tag="qk1")     # rows 0-63 qT, 64-127 kT
    klow = sb.tile([H, S], BF16, tag="klow")   # kT on partitions 0-63
    v_all = sb.tile([P, NT, H + 1], BF16, tag="v_all")  # [1 | v] per j-tile
    nc.vector.memset(v_all[:, :, 0:1], 1.0)

    at_pair = {}   # (b, pt) -> [128, 1024] tile: j-tiles 2pt | 2pt+1
    at_diag = {}   # (b, dj) -> [128, 512] tile

    def emit_pass1(nb, halves=False):
        spans = ((0, 2), (2, 4)) if halves else ((0, 4),)
        for t_lo, t_hi in spans:
            w = 128 * (t_hi - t_lo)
            c0 = 512 * nb + 128 * t_lo
            ps = psum.tile([P, 512], FP32, tag="proj", name=f"p1_{nb}_{t_lo}")
            for et in range(ET):
                nc.tensor.matmul(
                    ps[:, 0:w], wqk_all3[:, et, :],
                    xT_view[:, et, 4 * nb + t_lo:4 * nb + t_hi, :],
                    start=(et == 0), stop=(et == ET - 1),
                )
            nc.vector.tensor_copy(qk1[:, c0:c0 + w], ps[:, 0:w])
            nc.vector.tensor_copy(klow[:, c0:c0 + w], qk1[H:P, c0:c0 + w])

    def emit_v(t_lo, t_hi):
        for t in range(t_lo, t_hi):
            ps = psum.tile([P, 512], FP32, tag="proj", name=f"v_{t}")
            for et in range(ET):
                nc.tensor.matmul(
                    ps[:, 0:H], xT_view[:, et, t, :], wv_all3[:, et, :],
                    start=(et == 0), stop=(et == ET - 1),
                )
            nc.vector.tensor_copy(v_all[:, t, 1:], ps[:, 0:H])

    def emit_score_pairs(b):
        # full j-tile pairs: two K=64 matmuls into a 2-bank fp32 PSUM tile,
        # evicted by a single wide exp
        for pt in range(2 * b):
            ps = psum.tile([P, 1024], FP32, tag="sc", bufs=3, name=f"s{b}_{pt}")
            for h in range(2):
                t = 2 * pt + h
                nc.tensor.matmul(
                    ps[:, h * 512:(h + 1) * 512],
                    klow[:, ts(t, P)],
                    qk1[0:H, ts(b, 512)],
                    start=True, stop=True,
                )
            at = atp.tile([P, 1024], BF16, tag="pair", bufs=12,
                          name=f"a{b}_{pt}")
            nc.scalar.activation(
                at, ps, mybir.ActivationFunctionType.Exp, scale=0.125,
            )
            at_pair[(b, pt)] = at

    def emit_score_diags(b):
        # diagonal tiles: two per PSUM tile, narrowed width, own exp + mask
        for dh in range(2):
            ps = psum.tile([P, 1024], FP32, tag="sc", bufs=3, name=f"sd{b}_{dh}")
            for h in range(2):
                dj = 2 * dh + h
                t = 4 * b + dj
                c0 = P * dj
                nc.tensor.matmul(
                    ps[:, h * 512 + c0:h * 512 + 512], klow[:, ts(t, P)],
                    qk1[0:H, 512 * b + c0:512 * (b + 1)],
                    start=True, stop=True,
                )
            for h in range(2):
                dj = 2 * dh + h
                c0 = P * dj
                at = atp.tile(
                    [P, 512], BF16, tag="diag", bufs=16, name=f"ad{b}_{dj}"
                )
                nc.scalar.activation(
                    at[:, c0:512], ps[:, h * 512 + c0:h * 512 + 512],
                    mybir.ActivationFunctionType.Exp, scale=0.125,
                )
                nc.gpsimd.affine_select(
                    out=at[:, c0:c0 + P],
                    in_=at[:, c0:c0 + P],
                    compare_op=mybir.AluOpType.is_ge,
                    fill=0.0,
                    base=0,
                    pattern=[[1, P]],
                    channel_multiplier=-1,
                )
                at_diag[(b, dj)] = at

    def _at_slice(b, t, q):
        """attnT[j-tile t, i-quarter q of block b] as a [128, 128] lhsT."""
        if t < 4 * b:
            tile_ = at_pair[(b, t // 2)]
            return tile_[:, (t % 2) * 512 + q * P:(t % 2) * 512 + (q + 1) * P]
        return at_diag[(b, t - 4 * b)][:, q * P:(q + 1) * P]

    def emit_av(b, split_out=False):
        out_sb = fin.tile([P, 4, H], FP32, tag="osb", name=f"osb{b}")
        for q in range(4):
            n_t = 4 * b + q + 1  # causal: j-tiles 0 .. 4b+q
            ps = psum.tile([P, 512], FP32, tag="proj", name=f"av{b}_{q}")
            for t in range(n_t):
                nc.tensor.matmul(
                    ps[:, 0:H + 1], _at_slice(b, t, q), v_all[:, t, :],
                    start=(t == 0), stop=(t == n_t - 1),
                )
            r = fin.tile([P, 1], FP32, tag="recip", name=f"r{b}_{q}")
            nc.vector.reciprocal(r, ps[:, 0:1])
            nc.vector.tensor_scalar_mul(out_sb[:, q, :], ps[:, 1:H + 1], r)
            if split_out and q == 1:
                nc.sync.dma_start(
                    out=out_d[512 * b:512 * b + 256, :].rearrange(
                        "(q p) h -> p q h", p=P
                    ),
                    in_=out_sb[:, 0:2, :],
                )
            elif split_out and q >= 2:
                nc.sync.dma_start(
                    out=out_d[512 * b + 128 * q:512 * b + 128 * (q + 1), :],
                    in_=out_sb[:, q, :],
                )
        if not split_out:
            nc.sync.dma_start(
                out=out_d[ts(b, 512), :].rearrange("(q p) h -> p q h", p=P),
                in_=out_sb,
            )

    # --- main software-pipelined schedule ---------------------------------
    # prioritize the pass1 -> scores chain (ACT's exp feed); v/AV fill the
    # PE while ACT drains, diag exps stay last per block so AV can stagger
    emit_pass1(0, halves=True)
    emit_v(0, 2)
    emit_score_diags(0)
    emit_pass1(1)
    emit_score_pairs(1)
    emit_v(2, 4)
    emit_score_diags(1)
    emit_pass1(2)
    emit_score_pairs(2)
    emit_v(4, 8)
    emit_score_diags(2)
    emit_pass1(3)
    emit_score_pairs(3)
    emit_av(0)
    emit_score_diags(3)
    emit_v(8, 12)
    emit_v(12, 16)
    emit_av(1)
    emit_av(2)
    emit_av(3, split_out=True)


_NC_CACHE = {}


def _build_nc():
    if "nc" not in _NC_CACHE:
        from contextlib import ExitStack

        nc = bacc_mod.Bacc("TRN2")
        xt_d = nc.dram_tensor("xt", [P, ET * S], BF16, kind="ExternalInput")
        wqk_d = nc.dram_tensor("wqk", [P, ET * P], BF16, kind="ExternalInput")
        wv_d = nc.dram_tensor("wv", [P, ET * H], BF16, kind="ExternalInput")
        out_d = nc.dram_tensor("out", [S, H], FP32, kind="ExternalOutput")
        with tile.TileContext(nc) as tc:
            with ExitStack() as ctx:
                _emit(nc, tc, ctx, xt_d[:, :], wqk_d[:, :], wv_d[:, :],
                      out_d[:, :])
        nc.compile()
        _NC_CACHE["nc"] = nc
    return _NC_CACHE["nc"]


def _relayout_e_major(w):
    """[M, E] fp32 -> bf16 [128, ET*M]: out[p, e*M + m] = w[m, e*128 + p]."""
    m = w.shape[0]
    return np.ascontiguousarray(
        w.T.reshape(ET, P, m).transpose(1, 0, 2).reshape(P, ET * m)
    ).astype(ml_dtypes.bfloat16)


def kernel(x, w_q, w_k, w_v, _trace=False, _trace_kwargs=None):
    nc = _build_nc()
    x = np.ascontiguousarray(x, dtype=np.float32)
    # host-side layout permutations (no arithmetic): see module docstring
    wqk_host = _relayout_e_major(
        np.concatenate(
            [np.asarray(w_q, np.float32), np.asarray(w_k, np.float32)], axis=0
        )
    )
    wv_host = _relayout_e_major(np.asarray(w_v, np.float32))
    in_maps = []
    for b in range(N_CORES):
        # xt[p, e, t, s] = x[b][t*128+s, e*128+p], pre-cast to bf16
        xt = np.ascontiguousarray(
            x[b].reshape(NT, P, ET, P).transpose(3, 2, 0, 1).reshape(P, ET * S)
        ).astype(ml_dtypes.bfloat16)
        in_maps.append({"xt": xt, "wqk": wqk_host, "wv": wv_host})
    res = run_bass_kernel_spmd(
        nc, in_maps, list(range(N_CORES)), trace=_trace,
        **(_trace_kwargs or {}),
    )
    out = np.stack([res.results[b]["out"] for b in range(N_CORES)])
    if _trace:
        return out.astype(np.float32), res
    return out.astype(np.float32)


# revision 16
# speedup vs baseline: 1.2022x; 1.0107x over previous
"""Single-head causal attention kernel for Trainium2, 8-core data-parallel.

Problem: x[8, 2048, 1024], w_q/w_k/w_v[64, 1024] (torch Linear convention)
  q = x @ w_q.T; k = x @ w_k.T; v = x @ w_v.T          [B, S, H]
  out = softmax(mask(q @ k.T / sqrt(H))) @ v           [B, S, H]

Sharding: data-parallel over batch, one batch element per NeuronCore.
The host-side shard step also re-lays-out the tensors (pure permutation,
no arithmetic) so the device kernel needs no transposes at all:
  xT_host[p, e, t, s] = x[b][t*128+s, e*128+p]      -> [128, 16384] fp32
  wqk_host[p, e, m]   = concat(w_q, w_k)[m, e*128+p] -> [128, 1024] fp32
  wv_host[p, e, m]    = w_v[m, e*128+p]              -> [128, 512]  fp32

Per-core plan (S=2048, E=1024, H=64):
  - xT loaded fp32->bf16 (SWDGE cast) in a few chunked DMAs, already in
    the [p, e, t, s] transposed layout the matmuls need.
  - pass1 -> [qT; kT] packed (rows 0-63 = qT, 64-127 = kT) [128, 2048];
    kT duplicated onto partitions 0-63 (klow) for the score lhsT.
  - v computed in NATURAL layout [j, 64] via lhsT = xT blocks (M=128,
    N=64) straight into v_all[:, t, 1:65]; col 0 holds ones.
  - scoresT[j, i] = kT_t.T @ qT (K=64, N=512) -> fp32 PSUM; full j-tile
    pairs share a 2-bank [128, 1024] PSUM tile and get a single wide exp
    on ACT (1/8 softmax scale folded in); diagonal tiles get narrowed
    matmuls/exps plus gpsimd affine_select causal masking (fill 0).
  - AV in natural layout: o[i, {den,h}] = sum_t attnT_quarter.T @ v_aug
    (lhsT = attnT [j, 128-i-quarter], rhs = v_all[:, t, :] = [1 | v],
    M=128, N=65, fp32 PSUM accumulate). Column 0 gives the softmax
    denominator; normalize with reciprocal + tensor_scalar_mul on DVE.
  - A few zero matmuls at t=0 warm the PE p-state ramp before real work.
"""

import ml_dtypes
import numpy as np

import concourse.bass as bass
import concourse.bacc as bacc_mod
import concourse.tile as tile
from concourse import mybir
from concourse.bass import ts
from concourse.bass_utils import run_bass_kernel_spmd

B, S, E, H = 8, 2048, 1024, 64
P = 128
NB = S // 512          # 4 column blocks of 512
NT = S // P            # 16 row tiles of 128
ET = E // P            # 8 contraction tiles of 128
FP32 = mybir.dt.float32
BF16 = mybir.dt.bfloat16

N_CORES = 8

# --- schedule knobs -------------------------------------------------------
# HWDGE xT-load chunking: (first_tile, num_tiles)
X_CHUNKS = ((0, 2), (2, 2), (4, 2), (6, 2), (8, 2), (10, 2), (12, 2), (14, 2))
N_WARMUP = 7  # zero matmuls to bridge the PE p-state ramp until data


def _emit(nc, tc, ctx, xt_d, wqk_d, wv_d, out_d):
    consts = ctx.enter_context(tc.tile_pool(name="consts", bufs=1))
    sb = ctx.enter_context(tc.tile_pool(name="sb", bufs=1))
    atp = ctx.enter_context(tc.tile_pool(name="atp", bufs=1))
    fin = ctx.enter_context(tc.tile_pool(name="fin", bufs=4))
    psum = ctx.enter_context(tc.tile_pool(name="psum", bufs=2, space="PSUM"))

    # --- PE warmup: zero matmuls while DMAs are in flight ----------------
    wu = consts.tile([P, 512], BF16, tag="warm")
    nc.vector.memset(wu, 0.0)
    for i in range(N_WARMUP):
        ps = psum.tile([P, 512], FP32, tag="proj", name=f"wu{i}")
        nc.tensor.matmul(ps, wu[:, 0:P], wu, start=True, stop=True)

    # --- loads (HWDGE, bf16 pre-cast on host, layout pre-transposed) -----
    # order: first two x chunks, weights, remaining x chunks — minimizes
    # the critical chain into pass1(0)
    xT_all = sb.tile([P, ET * S], BF16, tag="xT_all")
    xT_view = xT_all.rearrange("p (e t s) -> p e t s", e=ET, t=NT)
    xt_dv = xt_d.rearrange("p (e t s) -> p e t s", e=ET, t=NT)

    def emit_x_chunk(t0, k):
        nc.sync.dma_start(
            out=xT_view[:, :, t0:t0 + k, :], in_=xt_dv[:, :, t0:t0 + k, :]
        )

    emit_x_chunk(*X_CHUNKS[0])
    wqk_all3 = sb.tile([P, ET, P], BF16, tag="wqk")
    nc.sync.dma_start(
        out=wqk_all3, in_=wqk_d.rearrange("p (e m) -> p e m", e=ET)
    )
    emit_x_chunk(*X_CHUNKS[1])
    wv_all3 = sb.tile([P, ET, H], BF16, tag="wv")
    nc.sync.dma_start(
        out=wv_all3, in_=wv_d.rearrange("p (e m) -> p e m", e=ET)
    )
    for t0, k in X_CHUNKS[2:]:
        emit_x_chunk(t0, k)

    # --- persistent SBUF tensors -----------------------------------------
    qk1 = sb.tile([P, S], BF16, tag="qk1")     # rows 0-63 qT, 64-127 kT
    klow = sb.tile([H, S], BF16, tag="klow")   # kT on partitions 0-63
    v_all = sb.tile([P, NT, H + 1], BF16, tag="v_all")  # [1 | v] per j-tile
    nc.vector.memset(v_all[:, :, 0:1], 1.0)

    at_pair = {}   # (b, pt) -> [128, 1024] tile: j-tiles 2pt | 2pt+1
    at_diag = {}   # (b, dj) -> [128, 512] tile

    def emit_pass1(nb, halves=False):
        spans = ((0, 2), (2, 4)) if halves else ((0, 4),)
        for t_lo, t_hi in spans:
            w = 128 * (t_hi - t_lo)
            c0 = 512 * nb + 128 * t_lo
            ps = psum.tile([P, 512], FP32, tag="proj", name=f"p1_{nb}_{t_lo}")
            for et in range(ET):
                nc.tensor.matmul(
                    ps[:, 0:w], wqk_all3[:, et, :],
                    xT_view[:, et, 4 * nb + t_lo:4 * nb + t_hi, :],
                    start=(et == 0), stop=(et == ET - 1),
                )
            nc.vector.tensor_copy(qk1[:, c0:c0 + w], ps[:, 0:w])
            nc.vector.tensor_copy(klow[:, c0:c0 + w], qk1[H:P, c0:c0 + w])

    def emit_v(t_lo, t_hi):
        for t in range(t_lo, t_hi):
            ps = psum.tile([P, 512], FP32, tag="proj", name=f"v_{t}")
            for et in range(ET):
                nc.tensor.matmul(
                    ps[:, 0:H], xT_view[:, et, t, :], wv_all3[:, et, :],
                    start=(et == 0), stop=(et == ET - 1),
                )
            nc.vector.tensor_copy(v_all[:, t, 1:], ps[:, 0:H])

    def emit_score_pairs(b):
        # full j-tile pairs: two K=64 matmuls into a 2-bank fp32 PSUM tile,
        # evicted by a single wide exp
        for pt in range(2 * b):
            ps = psum.tile([P, 1024], FP32, tag="sc", bufs=3, name=f"s{b}_{pt}")
            for h in range(2):
                t = 2 * pt + h
                nc.tensor.matmul(
                    ps[:, h * 512:(h + 1) * 512],
                    klow[:, ts(t, P)],
                    qk1[0:H, ts(b, 512)],
                    start=True, stop=True,
                )
            at = atp.tile([P, 1024], BF16, tag="pair", bufs=12,
                          name=f"a{b}_{pt}")
            nc.scalar.activation(
                at, ps, mybir.ActivationFunctionType.Exp, scale=0.125,
            )
            at_pair[(b, pt)] = at

    def emit_score_diags(b):
        # diagonal tiles: two per PSUM tile, narrowed width, own exp + mask
        for dh in range(2):
            ps = psum.tile([P, 1024], FP32, tag="sc", bufs=3, name=f"sd{b}_{dh}")
            for h in range(2):
                dj = 2 * dh + h
                t = 4 * b + dj
                c0 = P * dj
                nc.tensor.matmul(
                    ps[:, h * 512 + c0:h * 512 + 512], klow[:, ts(t, P)],
                    qk1[0:H, 512 * b + c0:512 * (b + 1)],
                    start=True, stop=True,
                )
            for h in range(2):
                dj = 2 * dh + h
                c0 = P * dj
                at = atp.tile(
                    [P, 512], BF16, tag="diag", bufs=16, name=f"ad{b}_{dj}"
                )
                nc.scalar.activation(
                    at[:, c0:512], ps[:, h * 512 + c0:h * 512 + 512],
                    mybir.ActivationFunctionType.Exp, scale=0.125,
                )
                nc.gpsimd.affine_select(
                    out=at[:, c0:c0 + P],
                    in_=at[:, c0:c0 + P],
                    compare_op=mybir.AluOpType.is_ge,
                    fill=0.0,
                    base=0,
                    pattern=[[1, P]],
                    channel_multiplier=-1,
                )
                at_diag[(b, dj)] = at

    def _at_slice(b, t, q):
        """attnT[j-tile t, i-quarter q of block b] as a [128, 128] lhsT."""
        if t < 4 * b:
            tile_ = at_pair[(b, t // 2)]
            return tile_[:, (t % 2) * 512 + q * P:(t % 2) * 512 + (q + 1) * P]
        return at_diag[(b, t - 4 * b)][:, q * P:(q + 1) * P]

    def emit_av(b, split_out=False):
        out_sb = fin.tile([P, 4, H], FP32, tag="osb", name=f"osb{b}")
        for q in range(4):
            n_t = 4 * b + q + 1  # causal: j-tiles 0 .. 4b+q
            ps = psum.tile([P, 512], FP32, tag="proj", name=f"av{b}_{q}")
            for t in range(n_t):
                nc.tensor.matmul(
                    ps[:, 0:H + 1], _at_slice(b, t, q), v_all[:, t, :],
                    start=(t == 0), stop=(t == n_t - 1),
                )
            r = fin.tile([P, 1], FP32, tag="recip", name=f"r{b}_{q}")
            nc.vector.reciprocal(r, ps[:, 0:1])
            nc.vector.tensor_scalar_mul(out_sb[:, q, :], ps[:, 1:H + 1], r)
            if split_out and q == 1:
                nc.sync.dma_start(
                    out=out_d[512 * b:512 * b + 256, :].rearrange(
                        "(q p) h -> p q h", p=P
                    ),
                    in_=out_sb[:, 0:2, :],
                )
            elif split_out and q >= 2:
                nc.sync.dma_start(
                    out=out_d[512 * b + 128 * q:512 * b + 128 * (q + 1), :],
                    in_=out_sb[:, q, :],
                )
        if not split_out:
            nc.sync.dma_start(
                out=out_d[ts(b, 512), :].rearrange("(q p) h -> p q h", p=P),
                in_=out_sb,
            )

    # --- main software-pipelined schedule ---------------------------------
    # prioritize the pass1 -> scores chain (ACT's exp feed); v/AV fill the
    # PE while ACT drains, diag exps stay last per block so AV can stagger
    emit_pass1(0, halves=True)
    emit_v(0, 2)
    emit_score_diags(0)
    emit_pass1(1)
    emit_score_pairs(1)
    emit_v(2, 4)
    emit_score_diags(1)
    emit_pass1(2)
    emit_score_pairs(2)
    emit_v(4, 8)
    emit_score_diags(2)
    emit_pass1(3)
    emit_score_pairs(3)
    emit_av(0)
    emit_v(8, 12)
    emit_v(12, 16)
    emit_av(1)
    emit_av(2)
    emit_score_diags(3)
    emit_av(3, split_out=True)


_NC_CACHE = {}


def _build_nc():
    if "nc" not in _NC_CACHE:
        from contextlib import ExitStack

        nc = bacc_mod.Bacc("TRN2")
        xt_d = nc.dram_tensor("xt", [P, ET * S], BF16, kind="ExternalInput")
        wqk_d = nc.dram_tensor("wqk", [P, ET * P], BF16, kind="ExternalInput")
        wv_d = nc.dram_tensor("wv", [P, ET * H], BF16, kind="ExternalInput")
        out_d = nc.dram_tensor("out", [S, H], FP32, kind="ExternalOutput")
        with tile.TileContext(nc) as tc:
            with ExitStack() as ctx:
                _emit(nc, tc, ctx, xt_d[:, :], wqk_d[:, :], wv_d[:, :],
                      out_d[:, :])
        nc.compile()
        _NC_CACHE["nc"] = nc
    return _NC_CACHE["nc"]


def _relayout_e_major(w):
    """[M, E] fp32 -> bf16 [128, ET*M]: out[p, e*M + m] = w[m, e*128 + p]."""
    m = w.shape[0]
    return np.ascontiguousarray(
        w.T.reshape(ET, P, m).transpose(1, 0, 2).reshape(P, ET * m)
    ).astype(ml_dtypes.bfloat16)


def kernel(x, w_q, w_k, w_v, _trace=False, _trace_kwargs=None):
    nc = _build_nc()
    x = np.ascontiguousarray(x, dtype=np.float32)
    # host-side layout permutations (no arithmetic): see module docstring
    wqk_host = _relayout_e_major(
        np.concatenate(
            [np.asarray(w_q, np.float32), np.asarray(w_k, np.float32)], axis=0
        )
    )
    wv_host = _relayout_e_major(np.asarray(w_v, np.float32))
    in_maps = []
    for b in range(N_CORES):
        # xt[p, e, t, s] = x[b][t*128+s, e*128+p], pre-cast to bf16
        xt = np.ascontiguousarray(
            x[b].reshape(NT, P, ET, P).transpose(3, 2, 0, 1).reshape(P, ET * S)
        ).astype(ml_dtypes.bfloat16)
        in_maps.append({"xt": xt, "wqk": wqk_host, "wv": wv_host})
    res = run_bass_kernel_spmd(
        nc, in_maps, list(range(N_CORES)), trace=_trace,
        **(_trace_kwargs or {}),
    )
    out = np.stack([res.results[b]["out"] for b in range(N_CORES)])
    if _trace:
        return out.astype(np.float32), res
    return out.astype(np.float32)
